# revision 25
# baseline (speedup 1.0000x reference)
"""Linformer-style linear attention on 8 Trainium2 NeuronCores.

Problem: B=32 heads of  softmax(Q @ (K^T E^T + e_b)/sqrt(d)) @ (F V + f_b)
with N=4096, D=128, Kp=256. Batch dim sharded 4-per-core across 8 cores.

Design notes:
 - All matmul operands are bf16 (PSUM accumulates in f32). Validated offline:
   norm rel err ~4.8e-3, scale-relative absmax ~6.5e-3 vs f32 reference.
 - Host pre-tiles every input so each DMA is fully contiguous per partition.
 - Scores are computed TRANSPOSED: ST[k, n] = K_proj[d,k].T @ QT[d,n], so the
   exp() output is already in [k, n] layout and slices directly as lhsT of the
   PV matmul -- no on-chip transposes anywhere.
 - Softmax skips max-subtraction (scores verified |S| <= ~7.05 on the actual
   inputs). Row sums come free from a ones column appended to V_proj.
 - Biases fold into the PE accumulation groups as rank-1 matmuls (seeded
   first with start=True).
 - Output ships unnormalized with the rowsum column; host does the divide.
 - Emission interleaves batch b+1's projection matmuls between batch b's
   attention blocks so the in-order PE stream always has dense work while
   ACT computes exp(); startup DMAs are chunked so PE starts early.
"""

import os
import numpy as np
import ml_dtypes

B, N, D, Kp = 32, 4096, 128, 256
NCORES = 8
BPC = B // NCORES  # batches per core
SCALE = 1.0 / float(np.sqrt(D))
NT128 = N // 128   # 32
NT512 = N // 512   # 8
KC = Kp // 128     # 2
bf16 = ml_dtypes.bfloat16

_cache = {}
_IDENT = np.eye(128, dtype=bf16)


def _build_nc(bpc=BPC, debug=False):
    import concourse.bacc as bacc
    import concourse.tile as tile
    import concourse.mybir as mybir

    dt = mybir.dt
    AF = mybir.ActivationFunctionType

    nc = bacc.Bacc("TRN2", target_bir_lowering=False, debug=debug)

    qt = nc.declare_dram_parameter("qt", [bpc, D, N], dt.bfloat16, isOutput=False)
    kt = nc.declare_dram_parameter("kt", [bpc, 128, N], dt.bfloat16, isOutput=False)
    vt = nc.declare_dram_parameter("vt", [bpc, 128, N], dt.bfloat16, isOutput=False)
    ewt = nc.declare_dram_parameter("ewt", [128, NT128 * Kp], dt.bfloat16, isOutput=False)
    fwt = nc.declare_dram_parameter("fwt", [128, NT128 * Kp], dt.bfloat16, isOutput=False)
    eb = nc.declare_dram_parameter("eb", [1, Kp], dt.bfloat16, isOutput=False)
    fb = nc.declare_dram_parameter("fb", [1, Kp], dt.bfloat16, isOutput=False)
    ident = nc.declare_dram_parameter("ident", [128, 128], dt.bfloat16, isOutput=False)
    # out[b, nt, p, t*129+j] = (j<128: unnormalized O; j==128: softmax rowsum)
    # for output row n = nt*512 + t*128 + p. Host divides and reorders.
    out = nc.declare_dram_parameter("out", [bpc, NT512, 128, 4 * (D + 1)], dt.float32, isOutput=True)

    with tile.TileContext(nc) as tc:
        with (
            tc.tile_pool(name="const", bufs=1) as cpool,
            tc.tile_pool(name="inq", bufs=3) as qpool,
            tc.tile_pool(name="ink", bufs=2) as kpool,
            tc.tile_pool(name="inv", bufs=2) as vpool,
            tc.tile_pool(name="kp", bufs=2) as kppool,
            tc.tile_pool(name="vext", bufs=4) as vextpool,
            tc.tile_pool(name="exp", bufs=6) as exppool,
            tc.tile_pool(name="osb", bufs=4) as opool,
            tc.tile_pool(name="ps_kp", bufs=1, space="PSUM") as ps_kp,
            tc.tile_pool(name="ps_vp", bufs=1, space="PSUM") as ps_vp,
            tc.tile_pool(name="ps_st", bufs=4, space="PSUM") as ps_st,
            tc.tile_pool(name="ps_o", bufs=2, space="PSUM") as ps_o,
        ):
            ones_sb = cpool.tile([1, 128], dt.bfloat16)
            nc.vector.memset(ones_sb[:, :], 1.0)
            eb_sb = cpool.tile([1, Kp], dt.bfloat16)
            nc.sync.dma_start(eb_sb[:, :], eb[:, :])
            fb_sb = cpool.tile([1, Kp], dt.bfloat16)
            nc.sync.dma_start(fb_sb[:, :], fb[:, :])
            ident_sb = cpool.tile([128, 128], dt.bfloat16)
            nc.sync.dma_start(ident_sb[:, :], ident[:, :])
            ewt_sb = cpool.tile([128, NT128 * Kp], dt.bfloat16)
            fwt_sb = cpool.tile([128, NT128 * Kp], dt.bfloat16)
            Wq = NT128 * Kp // 4

            state = {}

            def alloc_inputs(b):
                state[b] = {
                    "k": kpool.tile([128, N], dt.bfloat16, tag="k", name=f"k{b}"),
                    "q": qpool.tile([128, N], dt.bfloat16, tag="q", bufs=3, name=f"q{b}"),
                    "v": vpool.tile([128, N], dt.bfloat16, tag="v", name=f"v{b}"),
                }

            def emit_input_piece(b, piece, engine):
                """Spread one batch's input DMAs over 4 pieces (k, qt, v quarters)."""
                st = state[b]
                def dk(q):
                    engine.dma_start(st["k"][:, q * 1024:(q + 1) * 1024], kt[b][:, q * 1024:(q + 1) * 1024])
                def dq(h):
                    engine.dma_start(st["q"][:, h * 2048:(h + 1) * 2048], qt[b][:, h * 2048:(h + 1) * 2048])
                def dv(q):
                    engine.dma_start(st["v"][:, q * 1024:(q + 1) * 1024], vt[b][:, q * 1024:(q + 1) * 1024])
                if piece == 0:
                    dk(0); dk(1)
                elif piece == 1:
                    dk(2); dk(3); dq(0)
                elif piece == 2:
                    dv(0); dv(1); dq(1)
                else:
                    dv(2); dv(3)

            def emit_kp_chunk(b, i):
                """i in 0..7: 4 contraction chunks each; bias at i==0, copy at i==7."""
                st = state[b]
                if i == 0:
                    kp_ps = ps_kp.tile([128, Kp], dt.float32, tag="kp_ps")
                    st["kp_ps"] = kp_ps
                    nc.tensor.matmul(
                        kp_ps[:, :], lhsT=ones_sb[:, :], rhs=eb_sb[:, :],
                        start=True, stop=False,
                    )
                kp_ps = st["kp_ps"]
                for c in range(4 * i, 4 * i + 4):
                    nc.tensor.matmul(
                        kp_ps[:, :],
                        lhsT=st["k"][:, c * 128:(c + 1) * 128],
                        rhs=ewt_sb[:, c * Kp:(c + 1) * Kp],
                        start=False,
                        stop=(c == NT128 - 1),
                    )
                if i == 7:
                    kp_sb = kppool.tile([128, Kp], dt.bfloat16, tag="kp")
                    nc.vector.tensor_copy(kp_sb[:, :], kp_ps[:, :])
                    st["kp"] = kp_sb

            def emit_vp_chunk(b, i):
                """i in 0..7: V_projT[d,k] accumulated like K_proj (free=256,
                weight loads hidden), then PE-transposed per k-chunk."""
                st = state[b]
                if i == 0:
                    vp_ps = ps_vp.tile([128, Kp], dt.float32, tag="vpT")
                    st["vp_ps"] = vp_ps
                    nc.tensor.matmul(
                        vp_ps[:, :], lhsT=ones_sb[:, :], rhs=fb_sb[:, :],
                        start=True, stop=False,
                    )
                vp_ps = st["vp_ps"]
                for c in range(4 * i, 4 * i + 4):
                    nc.tensor.matmul(
                        vp_ps[:, :],
                        lhsT=st["v"][:, c * 128:(c + 1) * 128],
                        rhs=fwt_sb[:, c * Kp:(c + 1) * Kp],
                        start=False,
                        stop=(c == NT128 - 1),
                    )
                if i == 7:
                    vpT_sb = kppool.tile([128, Kp], dt.bfloat16, tag="vpT_sb", name=f"vpT{b}")
                    nc.vector.tensor_copy(vpT_sb[:, :], vp_ps[:, :])
                    for kc in range(KC):
                        tr_ps = ps_vp.tile([128, 128], dt.bfloat16, tag="vpT", name=f"tr{b}_{kc}")
                        nc.tensor.transpose(tr_ps[:, :], vpT_sb[:, kc * 128:(kc + 1) * 128], ident_sb[:, :])
                        vext = vextpool.tile([128, D + 1], dt.bfloat16, tag=f"vext{kc}")
                        nc.vector.tensor_copy(vext[:, 0:D], tr_ps[:, :])
                        nc.vector.memset(vext[:, D:D + 1], 1.0)
                        st.setdefault("vext", {})[kc] = vext

            def emit_st(b, nt, kc):
                st = state[b]
                st_ps = ps_st.tile([128, 512], dt.float32, tag=f"st{kc}", bufs=2)
                nc.tensor.matmul(
                    st_ps[:, :],
                    lhsT=st["kp"][:, kc * 128:(kc + 1) * 128],
                    rhs=st["q"][:, nt * 512:(nt + 1) * 512],
                    start=True,
                    stop=True,
                )
                ex = exppool.tile([128, 512], dt.bfloat16, tag=f"exp{kc}", bufs=3)
                nc.scalar.activation(ex[:, :], st_ps[:, :], AF.Exp, scale=SCALE)
                st.setdefault("exp", {})[(nt, kc)] = ex

            def emit_o(b, nt):
                st = state[b]
                out_sb = opool.tile([128, 4 * (D + 1)], dt.float32, tag="osb")
                for pair in range(2):
                    o_ps = ps_o.tile([128, 2 * (D + 1)], dt.float32, tag="o_ps")
                    for tt in range(2):
                        t = pair * 2 + tt
                        for kc in range(KC):
                            nc.tensor.matmul(
                                o_ps[:, tt * (D + 1):(tt + 1) * (D + 1)],
                                lhsT=st["exp"][(nt, kc)][:, t * 128:(t + 1) * 128],
                                rhs=st["vext"][kc][:, :],
                                start=(kc == 0),
                                stop=(kc == KC - 1),
                            )
                    nc.vector.tensor_copy(
                        out_sb[:, pair * 2 * (D + 1):(pair + 1) * 2 * (D + 1)],
                        o_ps[:, :],
                    )
                for kc in range(KC):
                    del st["exp"][(nt, kc)]
                # split across DMA queues; quarters for the final block so the
                # kernel-tail drain never waits on one long serial transfer
                nsplit = 4 if (b == bpc - 1 and nt == NT512 - 1) else 2
                step = 4 * (D + 1) // nsplit
                for s in range(nsplit):
                    nc.sync.dma_start(
                        out[b, nt][:, s * step:(s + 1) * step],
                        out_sb[:, s * step:(s + 1) * step],
                    )

            # ---- emission schedule ----
            # Startup: batch-0 inputs + weights interleaved on sync HWDGE in
            # consumption order (ewt/k quarters feed KP, fwt/v feed VP, qt last).
            alloc_inputs(0)
            st0 = state[0]
            for q in range(4):
                nc.sync.dma_start(ewt_sb[:, q * Wq:(q + 1) * Wq], ewt[:, q * Wq:(q + 1) * Wq])
                nc.sync.dma_start(st0["k"][:, q * 1024:(q + 1) * 1024], kt[0][:, q * 1024:(q + 1) * 1024])
            nc.sync.dma_start(st0["q"][:, 0:2048], qt[0][:, 0:2048])
            for q in range(4):
                nc.sync.dma_start(fwt_sb[:, q * Wq:(q + 1) * Wq], fwt[:, q * Wq:(q + 1) * Wq])
                nc.sync.dma_start(st0["v"][:, q * 1024:(q + 1) * 1024], vt[0][:, q * 1024:(q + 1) * 1024])
            nc.sync.dma_start(st0["q"][:, 2048:4096], qt[0][:, 2048:4096])
            for i in range(8):
                emit_kp_chunk(0, i)
            for i in range(8):
                emit_vp_chunk(0, i)
            # Steady state: all per-batch input and output DMAs issue from the
            # gpsimd engine in one deterministic interleaved stream so outputs
            # are never starved behind prefetch. Projections of batch b+1 fill
            # the PE stream during the second half of batch b's attention.
            for b in range(bpc):
                if b + 1 < bpc:
                    alloc_inputs(b + 1)
                emit_st(b, 0, 0)
                emit_st(b, 0, 1)
                # per-nt filler: projections of b+1 spread over nt 2..7
                # (kp chunk i needs k quarter i//2; vp chunk (kc,j) needs v qj)
                PROJ = {2: [("kp", 0), ("kp", 1)], 3: [("kp", 2), ("kp", 3)],
                        4: [("kp", 4), ("kp", 5), ("vp", 0), ("vp", 1)],
                        5: [("kp", 6), ("kp", 7), ("vp", 2), ("vp", 3)],
                        6: [("vp", 4), ("vp", 5)], 7: [("vp", 6), ("vp", 7)]}
                for nt in range(NT512):
                    if nt + 1 < NT512:
                        emit_st(b, nt + 1, 0)
                        emit_st(b, nt + 1, 1)
                    if b + 1 < bpc:
                        for kind, i in PROJ.get(nt, []):
                            (emit_kp_chunk if kind == "kp" else emit_vp_chunk)(b + 1, i)
                    emit_o(b, nt)
                    if b + 1 < bpc and nt < 4:
                        emit_input_piece(b + 1, nt, nc.sync)
                del state[b]

    nc.compile()
    return nc


def _prep(Q, K, V, E_W, E_b, F_W, F_b):
    """Host-side: cast to bf16 and pre-tile so every DMA is contiguous."""
    QT = np.ascontiguousarray(
        Q.astype(bf16).transpose(0, 2, 1))                      # [B, D, N]
    Kt = np.ascontiguousarray(
        K.astype(bf16).reshape(B, NT128, 128, D).transpose(0, 2, 1, 3)
    ).reshape(B, 128, N)
    Vt = np.ascontiguousarray(
        V.astype(bf16).reshape(B, NT128, 128, D).transpose(0, 2, 1, 3)
    ).reshape(B, 128, N)
    EWT = np.ascontiguousarray(
        E_W.T.astype(bf16).reshape(NT128, 128, Kp).transpose(1, 0, 2)
    ).reshape(128, NT128 * Kp)
    FWT = np.ascontiguousarray(
        F_W.T.astype(bf16).reshape(NT128, 128, Kp).transpose(1, 0, 2)
    ).reshape(128, NT128 * Kp)
    ebh = E_b.astype(bf16).reshape(1, Kp)
    fbh = F_b.astype(bf16).reshape(1, Kp)
    return QT, Kt, Vt, EWT, FWT, ebh, fbh


def _postprocess(raw):
    """raw [nb, NT512, 128, 4*(D+1)] f32 -> normalized O [nb, N, D]."""
    nb = raw.shape[0]
    r = raw.reshape(nb, NT512, 128, 4, D + 1)
    r = r.transpose(0, 1, 3, 2, 4)            # [nb, nt, t, p, D+1]
    r = r.reshape(nb, N, D + 1)
    return (r[:, :, :D] / r[:, :, D:D + 1]).astype(np.float32)


def kernel(Q, K, V, E_W, E_b, F_W, F_b):
    QT, Kt, Vt, EWT, FWT, ebh, fbh = _prep(Q, K, V, E_W, E_b, F_W, F_b)

    if "nc" not in _cache:
        _cache["nc"] = _build_nc()
    nc = _cache["nc"]

    in_maps = []
    for i in range(NCORES):
        sl = slice(i * BPC, (i + 1) * BPC)
        in_maps.append({
            "qt": QT[sl], "kt": Kt[sl], "vt": Vt[sl],
            "ewt": EWT, "fwt": FWT, "eb": ebh, "fb": fbh,
            "ident": _IDENT,
        })

    from concourse.bass_utils import run_bass_kernel_spmd

    res = run_bass_kernel_spmd(nc, in_maps, list(range(NCORES)))
    kernel.last_result = res
    kernel.last_exec_time_ns = res.exec_time_ns

    raw = np.concatenate(
        [np.asarray(res.results[i]["out"]) for i in range(NCORES)], axis=0
    )
    return np.ascontiguousarray(_postprocess(raw))


# revision 28
# speedup vs baseline: 1.1216x; 1.1216x over previous
"""Linformer-style linear attention on 8 Trainium2 NeuronCores.

Problem: B=32 heads of  softmax(Q @ (K^T E^T + e_b)/sqrt(d)) @ (F V + f_b)
with N=4096, D=128, Kp=256. Batch dim sharded 4-per-core across 8 cores.

Design notes:
 - All matmul operands are bf16 (PSUM accumulates in f32). Validated offline:
   norm rel err ~4.8e-3, scale-relative absmax ~6.5e-3 vs f32 reference.
 - Host pre-tiles every input so each DMA is fully contiguous per partition.
 - Scores are computed TRANSPOSED: ST[k, n] = K_proj[d,k].T @ QT[d,n], so the
   exp() output is already in [k, n] layout and slices directly as lhsT of the
   PV matmul -- no on-chip transposes anywhere.
 - Softmax skips max-subtraction (scores verified |S| <= ~7.05 on the actual
   inputs). Row sums come free from a ones column appended to V_proj.
 - Biases fold into the PE accumulation groups as rank-1 matmuls (seeded
   first with start=True).
 - Output ships unnormalized with the rowsum column; host does the divide.
 - Emission interleaves batch b+1's projection matmuls between batch b's
   attention blocks so the in-order PE stream always has dense work while
   ACT computes exp(); startup DMAs are chunked so PE starts early.
"""

import os
import numpy as np
import ml_dtypes

B, N, D, Kp = 32, 4096, 128, 256
NCORES = 8
BPC = B // NCORES  # batches per core
SCALE = 1.0 / float(np.sqrt(D))
NT128 = N // 128   # 32
NT512 = N // 512   # 8
KC = Kp // 128     # 2
bf16 = ml_dtypes.bfloat16

_cache = {}
_IDENT = np.eye(128, dtype=bf16)


def _build_nc(bpc=BPC, debug=False):
    import concourse.bacc as bacc
    import concourse.tile as tile
    import concourse.mybir as mybir

    dt = mybir.dt
    AF = mybir.ActivationFunctionType

    nc = bacc.Bacc("TRN2", target_bir_lowering=False, debug=debug)

    qt = nc.declare_dram_parameter("qt", [bpc, D, N], dt.bfloat16, isOutput=False)
    kt = nc.declare_dram_parameter("kt", [bpc, 128, N], dt.bfloat16, isOutput=False)
    vt = nc.declare_dram_parameter("vt", [bpc, 128, N], dt.bfloat16, isOutput=False)
    ewt = nc.declare_dram_parameter("ewt", [128, NT128 * Kp], dt.bfloat16, isOutput=False)
    fwt = nc.declare_dram_parameter("fwt", [128, NT128 * Kp], dt.bfloat16, isOutput=False)
    eb = nc.declare_dram_parameter("eb", [1, Kp], dt.bfloat16, isOutput=False)
    fb = nc.declare_dram_parameter("fb", [1, Kp], dt.bfloat16, isOutput=False)
    ident = nc.declare_dram_parameter("ident", [128, 128], dt.bfloat16, isOutput=False)
    # out[b, nt, p, t*129+j] = (j<128: unnormalized O; j==128: softmax rowsum)
    # for output row n = nt*512 + t*128 + p. Host divides and reorders.
    out = nc.declare_dram_parameter("out", [bpc, NT512, 128, 4 * (D + 1)], dt.float32, isOutput=True)

    with tile.TileContext(nc) as tc:
        with (
            tc.tile_pool(name="const", bufs=1) as cpool,
            tc.tile_pool(name="inq", bufs=3) as qpool,
            tc.tile_pool(name="ink", bufs=2) as kpool,
            tc.tile_pool(name="inv", bufs=2) as vpool,
            tc.tile_pool(name="kp", bufs=2) as kppool,
            tc.tile_pool(name="vext", bufs=4) as vextpool,
            tc.tile_pool(name="exp", bufs=6) as exppool,
            tc.tile_pool(name="osb", bufs=6) as opool,
            tc.tile_pool(name="ps_kp", bufs=1, space="PSUM") as ps_kp,
            tc.tile_pool(name="ps_vp", bufs=1, space="PSUM") as ps_vp,
            tc.tile_pool(name="ps_st", bufs=4, space="PSUM") as ps_st,
            tc.tile_pool(name="ps_o", bufs=2, space="PSUM") as ps_o,
        ):
            ones_sb = cpool.tile([1, 128], dt.bfloat16)
            nc.vector.memset(ones_sb[:, :], 1.0)
            eb_sb = cpool.tile([1, Kp], dt.bfloat16)
            nc.sync.dma_start(eb_sb[:, :], eb[:, :])
            fb_sb = cpool.tile([1, Kp], dt.bfloat16)
            nc.sync.dma_start(fb_sb[:, :], fb[:, :])
            ident_sb = cpool.tile([128, 128], dt.bfloat16)
            nc.sync.dma_start(ident_sb[:, :], ident[:, :])
            ewt_sb = cpool.tile([128, NT128 * Kp], dt.bfloat16)
            fwt_sb = cpool.tile([128, NT128 * Kp], dt.bfloat16)
            Wq = NT128 * Kp // 4

            state = {}

            def alloc_inputs(b):
                state[b] = {
                    "k": kpool.tile([128, N], dt.bfloat16, tag="k", name=f"k{b}"),
                    "q": qpool.tile([128, N], dt.bfloat16, tag="q", bufs=3, name=f"q{b}"),
                    "v": vpool.tile([128, N], dt.bfloat16, tag="v", name=f"v{b}"),
                }

            def emit_input_piece(b, piece, engine):
                """Spread one batch's input DMAs over 4 pieces (k, qt, v quarters)."""
                st = state[b]
                def dk(q):
                    engine.dma_start(st["k"][:, q * 1024:(q + 1) * 1024], kt[b][:, q * 1024:(q + 1) * 1024])
                def dq(h):
                    engine.dma_start(st["q"][:, h * 2048:(h + 1) * 2048], qt[b][:, h * 2048:(h + 1) * 2048])
                def dv(q):
                    engine.dma_start(st["v"][:, q * 1024:(q + 1) * 1024], vt[b][:, q * 1024:(q + 1) * 1024])
                if piece == 0:
                    dk(0); dk(1)
                elif piece == 1:
                    dk(2); dk(3); dq(0)
                elif piece == 2:
                    dv(0); dv(1); dq(1)
                else:
                    dv(2); dv(3)

            def emit_kp_chunk(b, i):
                """i in 0..7: 4 contraction chunks each; bias at i==0, copy at i==7."""
                st = state[b]
                if i == 0:
                    kp_ps = ps_kp.tile([128, Kp], dt.float32, tag="kp_ps")
                    st["kp_ps"] = kp_ps
                    nc.tensor.matmul(
                        kp_ps[:, :], lhsT=ones_sb[:, :], rhs=eb_sb[:, :],
                        start=True, stop=False,
                    )
                kp_ps = st["kp_ps"]
                for c in range(4 * i, 4 * i + 4):
                    nc.tensor.matmul(
                        kp_ps[:, :],
                        lhsT=st["k"][:, c * 128:(c + 1) * 128],
                        rhs=ewt_sb[:, c * Kp:(c + 1) * Kp],
                        start=False,
                        stop=(c == NT128 - 1),
                    )
                if i == 7:
                    kp_sb = kppool.tile([128, Kp], dt.bfloat16, tag="kp")
                    nc.vector.tensor_copy(kp_sb[:, :], kp_ps[:, :])
                    st["kp"] = kp_sb

            def emit_vp_chunk(b, i):
                """i in 0..7: kc = i//4, quarter j = i%4 (8 contraction chunks)."""
                st = state[b]
                kc, j = divmod(i, 4)
                if j == 0:
                    vp_ps = ps_vp.tile([128, 128], dt.float32, tag="vp_ps")
                    st["vp_ps"] = vp_ps
                    nc.tensor.matmul(
                        vp_ps[:, :], lhsT=fb_sb[:, kc * 128:(kc + 1) * 128],
                        rhs=ones_sb[:, :], start=True, stop=False,
                    )
                vp_ps = st["vp_ps"]
                for c in range(8 * j, 8 * j + 8):
                    nc.tensor.matmul(
                        vp_ps[:, :],
                        lhsT=fwt_sb[:, c * Kp + kc * 128: c * Kp + (kc + 1) * 128],
                        rhs=st["v"][:, c * 128:(c + 1) * 128],
                        start=False,
                        stop=(c == NT128 - 1),
                    )
                if j == 3:
                    vext = vextpool.tile([128, D + 1], dt.bfloat16, tag=f"vext{kc}")
                    nc.vector.tensor_copy(vext[:, 0:D], vp_ps[:, :])
                    nc.vector.memset(vext[:, D:D + 1], 1.0)
                    st.setdefault("vext", {})[kc] = vext

            def emit_st(b, nt, kc):
                st = state[b]
                st_ps = ps_st.tile([128, 512], dt.float32, tag=f"st{kc}", bufs=2)
                nc.tensor.matmul(
                    st_ps[:, :],
                    lhsT=st["kp"][:, kc * 128:(kc + 1) * 128],
                    rhs=st["q"][:, nt * 512:(nt + 1) * 512],
                    start=True,
                    stop=True,
                )
                ex = exppool.tile([128, 512], dt.bfloat16, tag=f"exp{kc}", bufs=4)
                nc.scalar.activation(ex[:, :], st_ps[:, :], AF.Exp, scale=SCALE)
                st.setdefault("exp", {})[(nt, kc)] = ex

            def emit_o(b, nt):
                st = state[b]
                out_sb = opool.tile([128, 4 * (D + 1)], dt.float32, tag="osb")
                for pair in range(2):
                    o_ps = ps_o.tile([128, 2 * (D + 1)], dt.float32, tag="o_ps")
                    for tt in range(2):
                        t = pair * 2 + tt
                        for kc in range(KC):
                            nc.tensor.matmul(
                                o_ps[:, tt * (D + 1):(tt + 1) * (D + 1)],
                                lhsT=st["exp"][(nt, kc)][:, t * 128:(t + 1) * 128],
                                rhs=st["vext"][kc][:, :],
                                start=(kc == 0),
                                stop=(kc == KC - 1),
                            )
                    nc.vector.tensor_copy(
                        out_sb[:, pair * 2 * (D + 1):(pair + 1) * 2 * (D + 1)],
                        o_ps[:, :],
                    )
                for kc in range(KC):
                    del st["exp"][(nt, kc)]
                # quarters for the final block only, so the kernel-tail drain
                # never waits on one long serial transfer
                nsplit = 4 if (b == bpc - 1 and nt == NT512 - 1) else 1
                step = 4 * (D + 1) // nsplit
                for s in range(nsplit):
                    nc.sync.dma_start(
                        out[b, nt][:, s * step:(s + 1) * step],
                        out_sb[:, s * step:(s + 1) * step],
                    )

            # ---- emission schedule ----
            # Startup: batch-0 inputs + weights interleaved on sync HWDGE in
            # consumption order (ewt/k quarters feed KP, fwt/v feed VP, qt last).
            alloc_inputs(0)
            st0 = state[0]
            for q in range(4):
                nc.sync.dma_start(ewt_sb[:, q * Wq:(q + 1) * Wq], ewt[:, q * Wq:(q + 1) * Wq])
                nc.sync.dma_start(st0["k"][:, q * 1024:(q + 1) * 1024], kt[0][:, q * 1024:(q + 1) * 1024])
            nc.sync.dma_start(st0["q"][:, 0:2048], qt[0][:, 0:2048])
            for q in range(4):
                nc.sync.dma_start(fwt_sb[:, q * Wq:(q + 1) * Wq], fwt[:, q * Wq:(q + 1) * Wq])
                nc.sync.dma_start(st0["v"][:, q * 1024:(q + 1) * 1024], vt[0][:, q * 1024:(q + 1) * 1024])
            nc.sync.dma_start(st0["q"][:, 2048:4096], qt[0][:, 2048:4096])
            for i in range(8):
                emit_kp_chunk(0, i)
            for i in range(8):
                emit_vp_chunk(0, i)
            # Steady state: all per-batch input and output DMAs issue from the
            # gpsimd engine in one deterministic interleaved stream so outputs
            # are never starved behind prefetch. Projections of batch b+1 fill
            # the PE stream during the second half of batch b's attention.
            for b in range(bpc):
                if b + 1 < bpc:
                    alloc_inputs(b + 1)
                emit_st(b, 0, 0)
                emit_st(b, 0, 1)
                # per-nt filler: projections of b+1 spread over nt 2..7
                # (kp chunk i needs k quarter i//2; vp chunk (kc,j) needs v qj)
                PROJ = {2: [("kp", 0), ("kp", 1)], 3: [("kp", 2), ("kp", 3)],
                        4: [("kp", 4), ("kp", 5), ("vp", 0), ("vp", 1)],
                        5: [("kp", 6), ("kp", 7), ("vp", 2), ("vp", 3)],
                        6: [("vp", 4), ("vp", 5)], 7: [("vp", 6), ("vp", 7)]}
                for nt in range(NT512):
                    if nt + 1 < NT512:
                        emit_st(b, nt + 1, 0)
                        emit_st(b, nt + 1, 1)
                    if b + 1 < bpc:
                        for kind, i in PROJ.get(nt, []):
                            (emit_kp_chunk if kind == "kp" else emit_vp_chunk)(b + 1, i)
                    emit_o(b, nt)
                    if b + 1 < bpc and nt < 4:
                        emit_input_piece(b + 1, nt, nc.sync)
                del state[b]

    nc.compile()
    return nc


def _prep(Q, K, V, E_W, E_b, F_W, F_b):
    """Host-side: cast to bf16 and pre-tile so every DMA is contiguous."""
    QT = np.ascontiguousarray(
        Q.astype(bf16).transpose(0, 2, 1))                      # [B, D, N]
    Kt = np.ascontiguousarray(
        K.astype(bf16).reshape(B, NT128, 128, D).transpose(0, 2, 1, 3)
    ).reshape(B, 128, N)
    Vt = np.ascontiguousarray(
        V.astype(bf16).reshape(B, NT128, 128, D).transpose(0, 2, 1, 3)
    ).reshape(B, 128, N)
    EWT = np.ascontiguousarray(
        E_W.T.astype(bf16).reshape(NT128, 128, Kp).transpose(1, 0, 2)
    ).reshape(128, NT128 * Kp)
    FWT = np.ascontiguousarray(
        F_W.T.astype(bf16).reshape(NT128, 128, Kp).transpose(1, 0, 2)
    ).reshape(128, NT128 * Kp)
    ebh = E_b.astype(bf16).reshape(1, Kp)
    fbh = F_b.astype(bf16).reshape(1, Kp)
    return QT, Kt, Vt, EWT, FWT, ebh, fbh


def _postprocess(raw):
    """raw [nb, NT512, 128, 4*(D+1)] f32 -> normalized O [nb, N, D]."""
    nb = raw.shape[0]
    r = raw.reshape(nb, NT512, 128, 4, D + 1)
    r = r.transpose(0, 1, 3, 2, 4)            # [nb, nt, t, p, D+1]
    r = r.reshape(nb, N, D + 1)
    return (r[:, :, :D] / r[:, :, D:D + 1]).astype(np.float32)


def kernel(Q, K, V, E_W, E_b, F_W, F_b):
    QT, Kt, Vt, EWT, FWT, ebh, fbh = _prep(Q, K, V, E_W, E_b, F_W, F_b)

    if "nc" not in _cache:
        _cache["nc"] = _build_nc()
    nc = _cache["nc"]

    in_maps = []
    for i in range(NCORES):
        sl = slice(i * BPC, (i + 1) * BPC)
        in_maps.append({
            "qt": QT[sl], "kt": Kt[sl], "vt": Vt[sl],
            "ewt": EWT, "fwt": FWT, "eb": ebh, "fb": fbh,
            "ident": _IDENT,
        })

    from concourse.bass_utils import run_bass_kernel_spmd

    res = run_bass_kernel_spmd(nc, in_maps, list(range(NCORES)))
    kernel.last_result = res
    kernel.last_exec_time_ns = res.exec_time_ns

    raw = np.concatenate(
        [np.asarray(res.results[i]["out"]) for i in range(NCORES)], axis=0
    )
    return np.ascontiguousarray(_postprocess(raw))
